# revision 13
# baseline (speedup 1.0000x reference)
"""Grouped-experts MoE FFN (SwiGLU) kernel for Trainium2, 8 NeuronCores.

Strategy: expert-parallel with host-side packing.  Token counts per expert
are data, so the host chops each expert's contiguous token block into
chunks and bins them into a uniform per-core "slot" structure
(S slots per core, compile-time sizes).  Every slot is bound to exactly
one expert; the expert's (host-pre-permuted) weights are plain kernel
inputs, so the SPMD program is identical on all 8 cores and needs no
device-side transposes or gather/scatter.

Weights and activations are fed as bf16 (PSUM accumulation stays fp32):
matmul streams at the same 1 col/cycle as fp32r, but weight-DMA traffic
halves and LDWEIGHTS gets the fast-weight-load path, which keeps the PE
fed from the first microsecond and avoids HAM re-throttle gaps.

Per-core device program, per slot of capacity L (tokens):
  phase A: for each 128-row h-chunk (22 of them):
      psum1[128,L] = sum_dc w1r[hc,dc].T @ xT[dc]     (bf16 matmuls)
      psum3[128,L] = sum_dc w3r[hc,dc].T @ xT[dc]
      h[hc] = silu(psum1) * psum3                     (ACT + DVE, bf16 out)
  phase B: for each 128-row d-chunk (8):
      po[128,L] = sum_hc w2r[dc,hc].T @ h[hc]
      DMA po -> outT[dc]                              ([D,L] fp32 out)

Host then transposes each slot's [D, L] output back and scatters into the
full [T, D] result (padding rows stay zero).
"""

import itertools
import numpy as np
from functools import lru_cache

E, D, H, T = 8, 1024, 2816, 16384
P = 128
DC, HC = D // P, H // P  # 8, 22
NCORES = 8

_FALLBACK = (1024, 1024, 1024)  # feasible for any counts with sum <= T


def _try_assign(Ls, counts, slack, node_budget=200_000):
    """DFS with a global padding budget: find per-expert chunk-count vectors
    (n per size class) such that every class uses <= NCORES chunks and the
    total padded capacity stays within `slack` of sum(counts).
    Returns list of per-expert vectors or None."""
    S = len(Ls)
    order = sorted(range(len(counts)), key=lambda i: -counts[i])
    asg = [(0,) * S] * len(counts)
    nodes = [0]

    def cands(c, budget, avail):
        res = []

        def rec(k, ns, cap):
            if cap >= c:
                if cap - c <= budget:
                    res.append((cap - c, tuple(ns) + (0,) * (S - len(ns))))
                return
            if k == S:
                return
            for n in range(avail[k] + 1):
                ns.append(n)
                rec(k + 1, ns, cap + n * Ls[k])
                ns.pop()
                if cap + n * Ls[k] >= c:
                    break

        rec(0, [], 0)
        res.sort()
        return res

    def dfs(j, used, budget):
        nodes[0] += 1
        if nodes[0] > node_budget:
            return False
        if j == len(order):
            return True
        i = order[j]
        if counts[i] == 0:
            return dfs(j + 1, used, budget)
        avail = [NCORES - u for u in used]
        for over, ns in cands(counts[i], budget, avail):
            asg[i] = ns
            if dfs(j + 1, tuple(u + n for u, n in zip(used, ns)), budget - over):
                return True
        return False

    return list(asg) if dfs(0, (0,) * S, slack) else None


@lru_cache(maxsize=None)
def _find_structure(counts):
    """Pick slot sizes minimizing total padded capacity (ties: fewer slots,
    larger minimum slot).  Sizes are multiples of 16 so per-expert padding
    can be nearly zero; every slot chunks into 512-col matmuls plus a
    trailing >=256 remainder (full-rate moving dim, PSUM-bank aligned)."""
    import time as _time
    counts = tuple(int(c) for c in counts)
    total = sum(counts)
    t_deadline = _time.monotonic() + 15.0
    best = None  # (cost, -minL, Ls, asg)
    for step, S in ((16, 4), (16, 3), (64, 5), (128, 3)):
        if _time.monotonic() > t_deadline and best is not None:
            break
        sizes = [s for s in range(256, 1025, step)
                 if s <= 512 or (s % 512 >= 256 or s % 512 == 0)]
        combos = sorted(itertools.combinations_with_replacement(sizes, S),
                        key=lambda L: sum(L))
        found = None
        for Ls in combos:
            ssum = sum(Ls)
            if 8 * ssum < total:
                continue
            if found and ssum > found[0]:
                break
            if best and ssum + 24 * (S - 3) >= best[0] + 64:
                break  # can't beat current best meaningfully
            if _time.monotonic() > t_deadline and (found or best):
                break
            asg = _try_assign(Ls, counts, 8 * ssum - total)
            if asg:
                cand = (ssum, Ls, asg)
                if not found or (ssum, -min(Ls)) < (found[0], -min(found[1])):
                    found = cand
        if found:
            cost = found[0] + 24 * (S - 3)
            key = (cost, -min(found[1]))
            if best is None or key < (best[0], best[1]):
                best = (cost, -min(found[1]), found[1], found[2])
    if best is not None:
        return best[2], best[3]
    return _FALLBACK, _try_assign(_FALLBACK, counts, 8 * sum(_FALLBACK) - total,
                                  node_budget=10_000_000)


def _make_plan(counts):
    """Return (Ls, chunks) where chunks[core][slot] = (expert, t0, n)."""
    Ls, asg = _find_structure(tuple(int(c) for c in counts))
    S = len(Ls)
    offs = np.concatenate([[0], np.cumsum(counts)]).astype(np.int64)
    per_class = [[] for _ in range(S)]
    for e, ns in enumerate(asg):
        pos = int(offs[e])
        remaining = int(counts[e])
        for k in sorted(range(S), key=lambda k: -Ls[k]):
            for _ in range(ns[k]):
                take = min(remaining, Ls[k])
                per_class[k].append((e, pos, take))
                pos += take
                remaining -= take
        assert remaining == 0
    chunks = [[None] * S for _ in range(NCORES)]
    for k in range(S):
        cl = per_class[k]
        assert len(cl) <= NCORES
        for j in range(NCORES):
            chunks[j][k] = cl[j] if j < len(cl) else (-1, 0, 0)
    # Device visit order: smallest slot first (fast startup: least xt bytes
    # before the first matmul), then the rest descending so the final
    # phase-B tail (last po copy + out DMA) is as small as possible.
    order = [0] + list(range(S - 1, 0, -1)) if S > 1 else [0]
    Ls = tuple(Ls[k] for k in order)
    chunks = [[row[k] for k in order] for row in chunks]
    return Ls, chunks


@lru_cache(maxsize=4)
def _build_program(Ls):
    import concourse.bacc as bacc
    import concourse.tile as tile
    from concourse import mybir

    f32 = mybir.dt.float32
    bf16 = mybir.dt.bfloat16
    nc = bacc.Bacc("TRN2", target_bir_lowering=False, debug=False,
                   num_devices=NCORES, name="moe_experts")

    xt_d, w1_d, w3_d, w2_d, out_d = [], [], [], [], []
    for s, L in enumerate(Ls):
        # (P, DC, L): whole tile is one DMA with DC*L*2-byte contiguous
        # per-partition lines (full HBM rate; per-dc pieces would have
        # L*2-byte lines and start up to 3x slower)
        xt_d.append(nc.dram_tensor(f"xt{s}", (P, DC, L), bf16, kind="ExternalInput"))
        w1_d.append(nc.dram_tensor(f"w1r{s}", (HC, P, DC, P), bf16, kind="ExternalInput"))
        w3_d.append(nc.dram_tensor(f"w3r{s}", (HC, P, DC, P), bf16, kind="ExternalInput"))
        w2_d.append(nc.dram_tensor(f"w2r{s}", (DC, P, HC, P), bf16, kind="ExternalInput"))
        out_d.append(nc.dram_tensor(f"out{s}", (DC, P, L), f32, kind="ExternalOutput"))

    def nchunks(L):
        # PSUM-bank-aligned matmul column chunks: 512s then a >=256 remainder
        out, n0 = [], 0
        while L - n0 > 512:
            out.append((n0, 512))
            n0 += 512
        if L - n0:
            assert 256 <= L - n0 <= 512, L
            out.append((n0, L - n0))
        return out

    with tile.TileContext(nc) as tc:
        with (
            tc.tile_pool(name="xpool", bufs=2) as xpool,
            tc.tile_pool(name="hpool", bufs=1) as hpool,
            tc.tile_pool(name="wpool", bufs=6) as wpool,
            tc.tile_pool(name="w2pool", bufs=3) as w2pool,
            tc.tile_pool(name="spool", bufs=2) as spool,
            tc.tile_pool(name="psum", bufs=2, space="PSUM") as psum,
        ):
            xts = [None] * len(Ls)

            def load_xt(s, L):
                xt = xpool.tile([P, DC, L], bf16, tag="xt")
                hh = DC // 2
                nc.sync.dma_start(xt[:, :hh, :], xt_d[s].ap()[:, :hh, :])
                nc.sync.dma_start(xt[:, hh:, :], xt_d[s].ap()[:, hh:, :])
                return xt

            # PE warm-up: dependency-free dummy matmuls bridge the initial
            # weight/token DMA wait so the HAM clock gate reaches 8/8 just
            # before the first real matmul (and stays there).
            warm_in = spool.tile([P, P], bf16, tag="warm")
            nc.vector.memset(warm_in[:], 0.0)
            warm_ps = psum.tile([P, P], f32, tag="p1")
            for _ in range(60):
                nc.tensor.matmul(warm_ps[:], warm_in[:], warm_in[:],
                                 start=True, stop=True)

            for s, L in enumerate(Ls):
                if s == 0:
                    xts[s] = load_xt(s, L)
                xt = xts[s]
                hbuf = hpool.tile([P, HC, L], bf16, tag="h")
                for hc in range(HC):
                    w1t = wpool.tile([P, DC, P], bf16, tag="w1")
                    nc.sync.dma_start(w1t[:], w1_d[s].ap()[hc])
                    w3t = wpool.tile([P, DC, P], bf16, tag="w3")
                    nc.sync.dma_start(w3t[:], w3_d[s].ap()[hc])
                    p1 = psum.tile([P, L], f32, tag="p1")
                    p3 = psum.tile([P, L], f32, tag="p3")
                    # dc outer / column-chunk inner: consecutive matmuls share
                    # the stationary weight tile
                    for dc in range(DC):
                        for (n0, nsz) in nchunks(L):
                            nc.tensor.matmul(
                                p1[:, n0:n0 + nsz],
                                w1t[:, dc, :],
                                xt[:, dc, n0:n0 + nsz],
                                start=(dc == 0), stop=(dc == DC - 1),
                            )
                    for dc in range(DC):
                        for (n0, nsz) in nchunks(L):
                            nc.tensor.matmul(
                                p3[:, n0:n0 + nsz],
                                w3t[:, dc, :],
                                xt[:, dc, n0:n0 + nsz],
                                start=(dc == 0), stop=(dc == DC - 1),
                            )
                    stmp = spool.tile([P, L], f32, tag="stmp")
                    nc.scalar.activation(stmp[:], p1[:], mybir.ActivationFunctionType.Silu)
                    nc.vector.tensor_mul(out=hbuf[:, hc, :], in0=stmp[:], in1=p3[:])
                for dc in range(DC):
                    if dc == 0 and s + 1 < len(Ls):
                        # prefetch next slot's tokens under this slot's phase B
                        xts[s + 1] = load_xt(s + 1, Ls[s + 1])
                    w2t = w2pool.tile([P, HC, P], bf16, tag="w2")
                    nc.sync.dma_start(w2t[:], w2_d[s].ap()[dc])
                    po = psum.tile([P, L], f32, tag="p1")
                    for hc in range(HC):
                        for (n0, nsz) in nchunks(L):
                            nc.tensor.matmul(
                                po[:, n0:n0 + nsz],
                                w2t[:, hc, :],
                                hbuf[:, hc, n0:n0 + nsz],
                                start=(hc == 0), stop=(hc == HC - 1),
                            )
                    ot = spool.tile([P, L], f32, tag="ot")
                    nc.any.tensor_copy(out=ot[:], in_=po[:])
                    nc.sync.dma_start(out_d[s].ap()[dc], ot[:])

    nc.compile()
    return nc


def _bf16(a):
    import ml_dtypes
    return np.asarray(a, dtype=ml_dtypes.bfloat16)


def _permute_w13(w):  # [H, D] -> [HC, P(k=d), DC, P(m=h)] bf16
    return np.ascontiguousarray(
        _bf16(w).reshape(HC, P, DC, P).transpose(0, 3, 2, 1))


def _permute_w2(w):  # [D, H] -> [DC, P(k=h), HC, P(m=d)] bf16
    return np.ascontiguousarray(
        _bf16(w).reshape(DC, P, HC, P).transpose(0, 3, 2, 1))


def kernel(x, w1, w2, w3, num_tokens_per_expert):
    import ml_dtypes
    from concourse.bass_utils import run_bass_kernel_spmd

    x = np.asarray(x, dtype=np.float32)
    w1 = np.asarray(w1, dtype=np.float32)
    w2 = np.asarray(w2, dtype=np.float32)
    w3 = np.asarray(w3, dtype=np.float32)
    counts = np.asarray(num_tokens_per_expert).astype(np.int64)

    Ls, chunks = _make_plan(counts)
    nc = _build_program(tuple(Ls))

    experts_used = sorted({e for row in chunks for (e, _, _) in row if e >= 0})
    if not experts_used:
        experts_used = [0]
    w1r = {e: _permute_w13(w1[e]) for e in experts_used}
    w3r = {e: _permute_w13(w3[e]) for e in experts_used}
    w2r = {e: _permute_w2(w2[e]) for e in experts_used}
    e_dummy = experts_used[0]

    xb = _bf16(x)
    in_maps = []
    for c in range(NCORES):
        m = {}
        for s, L in enumerate(Ls):
            e, t0, n = chunks[c][s]
            if e < 0:
                e = e_dummy
            xs = np.zeros((L, D), dtype=ml_dtypes.bfloat16)
            if n:
                xs[:n] = xb[t0:t0 + n]
            m[f"xt{s}"] = np.ascontiguousarray(
                xs.reshape(L, DC, P).transpose(2, 1, 0))
            m[f"w1r{s}"] = w1r[e]
            m[f"w3r{s}"] = w3r[e]
            m[f"w2r{s}"] = w2r[e]
        in_maps.append(m)

    res = run_bass_kernel_spmd(nc, in_maps, core_ids=list(range(NCORES)))

    out = np.zeros((T, D), dtype=np.float32)
    for c in range(NCORES):
        for s in range(len(Ls)):
            e, t0, n = chunks[c][s]
            if e < 0 or n == 0:
                continue
            o = res.results[c][f"out{s}"]  # [DC, P, L]
            out[t0:t0 + n] = o[:, :, :n].transpose(2, 0, 1).reshape(n, D)
    return out
